# revision 1
# baseline (speedup 1.0000x reference)
"""Distributed GCNConv kernel for Trainium2 (8 NeuronCores).

Graph-partition (expert-style) sharding by destination node: core k owns
destination rows [k*num_owned/8, (k+1)*num_owned/8). Edges whose dst lies
outside every core's owned range (row >= num_owned) are dropped (the
reference discards those aggregates).

Per core (SPMD, one NEFF):
  phase A: h = (deg * x) @ W for ALL nodes, replicated (for a random
    graph the halo is ~everything, so recomputing the 3.3 GFLOP matmul
    beats halo exchange). h is stored bf16 in a partition-major HBM table
    (row of node n = (n%128)*TA + n//128) so the store is one contiguous
    run per partition.
  phase B: edges sorted by (dst supertile, 32K source window, dst tile);
    per-(tile,window) slot counts are padded only to the max across cores
    (SPMD uniformity). gpsimd.dma_gather (int16 window-relative indices,
    <=2048/call, single_packet=False - larger calls overflow the SWDGE
    ring) pulls 256B messages; DVE builds a one-hot S per 128-edge chunk
    via tensor_scalar((iota + 128*v) == rowloc) where v selects the dst
    tile for chunks that span tile boundaries; PE accumulates
    psum[dst,feat] += S^T @ msgs with one PSUM bank per dst tile (the HW
    zeroes accumulation state at bank granularity); a final DVE pass
    applies deg_dst (+bias) and stores partition-major (unpermuted on
    host).

Self-contained: only needs numpy / ml_dtypes / the concourse Bass stack.
"""

import numpy as np
import ml_dtypes

import concourse.bacc as bacc
import concourse.bass as bass
import concourse.mybir as mybir
import concourse.tile as tile
from concourse.bass_utils import run_bass_kernel_spmd

P = 128
N_CORES = 8
WIN = 32768  # int16 gather-index window (dma_gather idxs are int16)
ST = 8       # dst tiles per supertile (one PSUM bank per tile, 8 banks)
GA = 32      # node tiles per phase-A block (1 MiB DMAs)
GMAX = 2048  # max gather indices per dma_gather call (SWDGE ring limit)
BF16 = mybir.dt.bfloat16
F32 = mybir.dt.float32
I16 = mybir.dt.int16
npbf16 = ml_dtypes.bfloat16
PAD_ROWLOC = -1.0  # one-hot miss marker (never equals iota + 128*v >= 0)


def _wrap_idx(a):
    """dma_gather index layout: linear index i lives at [i%16, i//16],
    replicated across the 8 Q7 cores -> [128, len//16]."""
    return np.tile(a.reshape(-1, 16).T, (8, 1))


def _plan(row, col, n_local, n_owned):
    """Host-side graph partitioning. Returns the SPMD-uniform schedule and
    the per-core packed index/rowloc arrays."""
    own = n_owned // N_CORES
    T = -(-own // P)                    # dst tiles per core
    S = -(-T // ST)                     # supertiles per core
    NW = -(-n_local // WIN)             # gather windows
    n_pad = -(-n_local // P) * P        # padded node count (phase A)

    TA = n_pad // P

    row = np.asarray(row).astype(np.int64)
    col = np.asarray(col).astype(np.int64)
    keep = row < n_owned
    r, c = row[keep], col[keep]
    core = r // own
    rl = r - core * own
    t = rl // P
    # the h table is stored partition-major (table row of node n is
    # (n%128)*TA + n//128) so the phase-A store is one long contiguous
    # run per partition; gather indices use table rows
    c = (c % P) * TA + (c // P)
    w = c // WIN

    # slot counts per (core, t, w); pad only to the max across cores (the
    # program must be SPMD-uniform). Runs pack back-to-back UNALIGNED; a
    # 128-edge chunk that spans tile boundaries issues one matmul per
    # covered tile, with the one-hot built against an offset iota slice.
    key = (core * T + t) * NW + w
    counts = np.bincount(key, minlength=N_CORES * T * NW).reshape(N_CORES, T, NW)
    C = counts.max(axis=0).astype(np.int64)  # [T, NW]
    assert counts.sum(axis=(0, 2)).min() > 0, "empty dst tile"

    # run order: supertile-major, then window, then tile (so each (s,w) is
    # one contiguous same-window gather call)
    runs = []   # (s, w, t, slot_off)
    calls = []  # (s, w, slot_off, L, tail) per (s,w), L 128-aligned; the
                # last `tail` slots are common pad (gathered as idx -1,
                # which the DMA skips)
    jobs = {}   # (s, w) -> list of (chunk_local, tile, iota_variant)
    first_job = {}  # tile -> (w, chunk_local, tile)
    last_job = {}
    off = 0
    t0_of_chunk = []  # global chunk -> first covered tile
    for s in range(S):
        ts = range(s * ST, min((s + 1) * ST, T))
        for wi in range(NW):
            call_off = off
            run_list = []
            for ti in ts:
                if C[ti, wi]:
                    runs.append((s, wi, ti, off))
                    run_list.append((ti, off, int(C[ti, wi])))
                    off += int(C[ti, wi])
            off_real = off
            L = -(-(off - call_off) // P) * P
            if not L:
                continue
            off = call_off + L
            calls.append((s, wi, call_off, L, call_off + L - off_real))
            nchk = L // P
            t0 = [None] * nchk
            jlist = []
            for (ti, o_r, cnt) in run_list:
                k_first = (o_r - call_off) // P
                k_last = (o_r + cnt - 1 - call_off) // P
                for k in range(k_first, k_last + 1):
                    if t0[k] is None:
                        t0[k] = ti
                    jlist.append((k, ti))
            jlist.sort()
            jlist2 = []
            for (k, ti) in jlist:
                v = ti - t0[k]
                assert 0 <= v < 64, (ti, t0[k])
                jlist2.append((k, ti, v))
                jk = (wi, k, ti)
                if ti not in first_job:
                    first_job[ti] = jk
                last_job[ti] = jk
            jobs[(s, wi)] = jlist2
            t0_of_chunk.extend(ti if ti is not None else 0 for ti in t0)
    TOT = off
    NCHK = TOT // P
    T0g = np.asarray(t0_of_chunk, dtype=np.int64)
    assert len(T0g) == NCHK

    # per-core slot packing
    per_core = []
    for k in range(N_CORES):
        m = core == k
        ck, rlk, tk, wk = c[m], rl[m], t[m], w[m]
        # composite key in stream order: (supertile, window, tile)
        g = (tk // ST) * (NW * T) + wk * T + tk
        order = np.argsort(g, kind="stable")
        ck, rlk, tk, wk, g = ck[order], rlk[order], tk[order], wk[order], g[order]
        ngrp = S * NW * T
        cnt = np.bincount(g, minlength=ngrp)
        starts = np.concatenate(([0], np.cumsum(cnt)))[:-1]
        rank = np.arange(len(g)) - starts[g]
        # map stream-order key -> run slot offset
        lut = np.full(ngrp, -1, dtype=np.int64)
        for (si, wi, ti, o) in runs:
            lut[si * (NW * T) + wi * T + ti] = o
        dest = lut[g] + rank
        assert dest.min() >= 0

        gidx = np.zeros(TOT, dtype=np.int16)
        rloc = np.full(TOT, PAD_ROWLOC, dtype=np.float32)
        gidx[dest] = (ck - wk * WIN).astype(np.int16)
        # rowloc relative to the chunk's FIRST covered tile: tiles later in
        # the chunk land at +128/+256 (matched by the offset iota slices)
        rloc[dest] = (rlk - tk * P + P * (tk - T0g[dest // P])).astype(np.float32)

        idx_wrapped = np.concatenate(
            [_wrap_idx(gidx[o:o + L]) for (_, _, o, L, _) in calls], axis=1
        )
        RL = np.ascontiguousarray(rloc.reshape(NCHK, P).T)
        per_core.append((idx_wrapped, RL))

    plan = dict(
        n_local=n_local, n_owned=n_owned, own=own, T=T, S=S, NW=NW,
        n_pad=n_pad, C=C, runs=runs, calls=calls, jobs=jobs, TOT=TOT,
        NCHK=NCHK, first_job=first_job, last_job=last_job,
    )
    return plan, per_core


def _build(plan, bias_zero=False):
    """Emit the Bass/Tile program (identical for all cores)."""
    n_pad, T, S, NW = plan["n_pad"], plan["T"], plan["S"], plan["NW"]
    C, calls, jobs = plan["C"], plan["calls"], plan["jobs"]
    TOT, NCHK = plan["TOT"], plan["NCHK"]
    first_job, last_job = plan["first_job"], plan["last_job"]
    TA = n_pad // P  # phase-A node tiles

    nc = bacc.Bacc("TRN2", target_bir_lowering=False, debug=False,
                   enable_asserts=False, num_devices=N_CORES)

    xt = nc.dram_tensor("xt", [P, n_pad], BF16, kind="ExternalInput")
    wgt = nc.dram_tensor("wgt", [P, P], BF16, kind="ExternalInput")
    dega = nc.dram_tensor("dega", [P, TA], F32, kind="ExternalInput")
    iot = nc.dram_tensor("iot", [P, P], BF16, kind="ExternalInput")
    biasb = nc.dram_tensor("biasb", [P, P], F32, kind="ExternalInput")
    idx = nc.dram_tensor("idx", [P, TOT // 16], I16, kind="ExternalInput")
    rld = nc.dram_tensor("rl", [P, NCHK], F32, kind="ExternalInput")
    degd = nc.dram_tensor("degd", [P, T], F32, kind="ExternalInput")
    out = nc.dram_tensor("out", [T * P, P], F32, kind="ExternalOutput")
    hst = nc.dram_tensor("h_stash", [n_pad, P], BF16, kind="Internal")

    with tile.TileContext(nc) as tc:
        with (
            tc.tile_pool(name="const", bufs=1) as constp,
            tc.tile_pool(name="xtp", bufs=2) as xtp,
            tc.tile_pool(name="hsb", bufs=2) as hsb,
            # one shared PSUM pool: every tile is one full bank (the HW
            # zeroes accumulation state at 2KB-bank granularity, so each
            # accumulator group must own its bank)
            tc.tile_pool(name="ps", bufs=8, space="PSUM") as psp,
            tc.tile_pool(name="msgs", bufs=4) as msgsp,
            tc.tile_pool(name="sone", bufs=12) as sonep,
            tc.tile_pool(name="osb", bufs=2) as osb,
        ):
            # resident constants / metadata
            w_sb = constp.tile([P, P], BF16)
            nc.sync.dma_start(w_sb[:], wgt[:, :])
            dega_sb = constp.tile([P, TA], F32)
            nc.sync.dma_start(dega_sb[:], dega[:, :])
            iot_sb = constp.tile([P, P], BF16)
            nc.sync.dma_start(iot_sb[:], iot[:, :])
            bias_sb = constp.tile([P, P], F32)
            nc.sync.dma_start(bias_sb[:], biasb[:, :])
            idx_sb = constp.tile([P, TOT // 16], I16)
            nc.sync.dma_start(idx_sb[:], idx[:, :])
            rl_sb = constp.tile([P, NCHK], F32)
            nc.sync.dma_start(rl_sb[:], rld[:, :])
            degd_sb = constp.tile([P, T], F32)
            nc.sync.dma_start(degd_sb[:], degd[:, :])

            # ---- phase A: h = (deg * x) @ W, stored bf16 node-major ----
            for b0 in range(0, TA, GA):
                nt = min(GA, TA - b0)
                xt_t = xtp.tile([P, GA * P], BF16, tag="xt")
                nc.sync.dma_start(xt_t[:, :nt * P],
                                  xt[:, b0 * P:(b0 + nt) * P])
                h_t = hsb.tile([P, GA * P], BF16, tag="h")
                for j0 in range(0, nt, 4):
                    nj = min(4, nt - j0)
                    ps = psp.tile([P, 512], F32, tag="ps")
                    for cc in range(nj):
                        nc.tensor.matmul(
                            ps[:, cc * P:(cc + 1) * P],
                            xt_t[:, (j0 + cc) * P:(j0 + cc + 1) * P],
                            w_sb[:],
                        )
                    for cc in range(nj):
                        tcol = b0 + j0 + cc
                        if cc % 2 == 0:
                            nc.vector.tensor_scalar(
                                h_t[:, (j0 + cc) * P:(j0 + cc + 1) * P],
                                ps[:, cc * P:(cc + 1) * P],
                                dega_sb[:, tcol:tcol + 1], None,
                                mybir.AluOpType.mult,
                            )
                        else:
                            nc.scalar.activation(
                                h_t[:, (j0 + cc) * P:(j0 + cc + 1) * P],
                                ps[:, cc * P:(cc + 1) * P],
                                mybir.ActivationFunctionType.Copy,
                                scale=dega_sb[:, tcol:tcol + 1],
                            )
                hv = hst[:, :].rearrange("(p j) f -> p j f", p=P)[:, b0:b0 + nt, :]
                nc.sync.dma_start(
                    hv, h_t[:, :nt * P].rearrange("p (j f) -> p j f", f=P))

            # ---- phase B: gather + one-hot matmul segment-sum ----
            max_chunks = max(L for (_, _, _, L, _) in calls) // P

            for s in range(S):
                ts0 = s * ST
                nts = min(ST, T - ts0)
                # one PSUM bank per dst tile; the group stays open across
                # all gather windows of this supertile
                pbs = [psp.tile([P, P], F32, tag="ps", name=f"pb{s}_{i}")
                       for i in range(nts)]
                for (cs, wi, o, L, tail) in calls:
                    if cs != s:
                        continue
                    nchk = L // P
                    wbase = wi * WIN
                    wsz = min(WIN, n_pad - wbase)
                    mg = msgsp.tile([P, max_chunks * P], BF16, tag="mg")
                    mg3 = mg[:, :nchk * P].rearrange("p (k f) -> p k f", f=P)
                    # the SWDGE ring can't take much over 8K descriptors in
                    # one self-triggered gather; sub-split large calls
                    for a in range(0, L, GMAX):
                        b = min(a + GMAX, L)
                        nc.gpsimd.dma_gather(
                            mg3[:, a // P:b // P, :],
                            hst[wbase:wbase + wsz, :],
                            idx_sb[:, (o + a) // 16:(o + b) // 16],
                            b - a, b - a, P,
                            single_packet=False,
                        )
                    kbase = o // P  # calls are 128-aligned
                    for (k, rt, v) in jobs[(s, wi)]:
                        tl = rt - ts0
                        kk = kbase + k
                        S_t = sonep.tile([P, P], BF16, tag="S")
                        # S = ((iota + 128*v) == rowloc); the ALU runs in
                        # fp32, so offset iota values stay exact
                        nc.vector.tensor_scalar(
                            S_t[:], iot_sb[:],
                            float(v * P), rl_sb[:, kk:kk + 1],
                            mybir.AluOpType.add,
                            mybir.AluOpType.is_equal,
                        )
                        nc.tensor.matmul(
                            pbs[tl][:],
                            S_t[:], mg3[:, k, :],
                            start=(first_job[rt] == (wi, k, rt)),
                            stop=(last_job[rt] == (wi, k, rt)),
                        )
                # evacuate supertile: out = psum * deg_dst + bias
                # (on ACT - keeps DVE free for the one-hot builds)
                ot = osb.tile([P, ST * P], F32, tag="ot")
                for tl in range(nts):
                    tg = ts0 + tl
                    nc.scalar.activation(
                        ot[:, tl * P:(tl + 1) * P],
                        pbs[tl][:],
                        mybir.ActivationFunctionType.Copy,
                        scale=degd_sb[:, tg:tg + 1],
                    )
                    if not bias_zero:
                        nc.vector.tensor_tensor(
                            ot[:, tl * P:(tl + 1) * P],
                            ot[:, tl * P:(tl + 1) * P],
                            bias_sb[:], mybir.AluOpType.add,
                        )
                ov = out[:, :].rearrange("(p j) f -> p j f", p=P)[:, ts0:ts0 + nts, :]
                nc.sync.dma_start(
                    ov, ot[:, :nts * P].rearrange("p (j f) -> p j f", f=P))

    nc.compile()
    return nc


def _pack_shared(x, weight, bias, deg, plan):
    n_local, n_pad, TA = plan["n_local"], plan["n_pad"], plan["n_pad"] // P
    xp = np.zeros((n_pad, P), dtype=np.float32)
    xp[:n_local] = x
    xt = np.ascontiguousarray(xp.T.astype(npbf16))
    wb = weight.astype(npbf16)
    dg = np.zeros((TA * P,), dtype=np.float32)
    dg[:n_local] = deg
    dega = np.ascontiguousarray(dg.reshape(TA, P).T)
    iot = np.ascontiguousarray(
        np.broadcast_to(np.arange(P, dtype=np.float32), (P, P))).astype(npbf16)
    biasb = np.ascontiguousarray(
        np.broadcast_to(bias.astype(np.float32), (P, P)))
    return xt, wb, dega, iot, biasb


def _pack_degd(deg, plan, k):
    own, T, n_local = plan["own"], plan["T"], plan["n_local"]
    d = np.zeros((T * P,), dtype=np.float32)
    n = min(own, n_local - k * own)
    d[:n] = deg[k * own:k * own + n]
    return np.ascontiguousarray(d.reshape(T, P).T)


_CACHE = {}


def _unpack_out(arr, plan):
    # out rows are partition-major (row = p*T + t); undo on host
    T, own = plan["T"], plan["own"]
    return np.asarray(arr).reshape(P, T, P).transpose(1, 0, 2).reshape(T * P, P)[:own]


def kernel(x, weight, bias, deg_inv_sqrt, row, col, num_owned,
           _want_trace=False):
    n_local = int(x.shape[0])
    n_owned = int(num_owned)
    x = np.asarray(x, dtype=np.float32)
    weight = np.asarray(weight, dtype=np.float32)
    bias = np.asarray(bias, dtype=np.float32)
    deg = np.asarray(deg_inv_sqrt, dtype=np.float32)

    plan, per_core = _plan(row, col, n_local, n_owned)
    xt, wb, dega, iot, biasb = _pack_shared(x, weight, bias, deg, plan)

    bias_zero = bool(np.all(bias == 0.0))
    sig = (n_local, n_owned, plan["TOT"], plan["C"].tobytes(), bias_zero)
    if sig in _CACHE:
        nc = _CACHE[sig]
    else:
        nc = _build(plan, bias_zero=bias_zero)
        _CACHE[sig] = nc

    in_maps = []
    for k in range(N_CORES):
        idxk, rlk = per_core[k]
        in_maps.append(dict(
            xt=xt, wgt=wb, dega=dega, iot=iot, biasb=biasb,
            idx=np.ascontiguousarray(idxk), rl=rlk,
            degd=_pack_degd(deg, plan, k),
        ))

    res = run_bass_kernel_spmd(nc, in_maps, core_ids=list(range(N_CORES)),
                               trace=_want_trace)

    own, T = plan["own"], plan["T"]
    full = np.empty((n_owned, P), dtype=np.float32)
    for k in range(N_CORES):
        full[k * own:(k + 1) * own] = _unpack_out(res.results[k]["out"], plan)
    kernel.last_results = res
    return full

